# revision 1
# baseline (speedup 1.0000x reference)
"""Trainium2 Bass kernel for nn_DistortionAttention.

Strategy: pure data parallel over (sample, row-half): core = 2*b + half.
Each core computes, for its sample b and its 2048-position slice of the
64x64 grid: the distortion classifier (redundantly per pair), the routed
spatial map (all 4 maps blended by a one-hot of the argmax branch), and
the non-local attention output for its positions.

Attention is computed transposed: S^T[m, n] = sum_c k[c,m] q[c,n] via
k-slices as matmul weights, exp on ScalarE (no max subtraction; softmax
denominator comes from a ones-row appended to v), and out' = v'^T-weights
@ exp(S^T) accumulated over m-tiles in PSUM.  Engine instruction streams
are static and in-order, so classifier/fft/map work is EMITTED woven into
the attention loop (generator fillers) to hide the S->exp->MM2 latency
and keep the PE warm.  S/exp run double-wide ([128,1024] = 2 PSUM banks)
to halve cross-engine sync hops.  fft2 is DFT matmuls with fftshift
folded into the host-precomputed DFT matrices.

All ScalarE transcendentals stay in one ACT table set
(natural_log_exp_and_others): sigmoid = 1/(1+exp(-z)) with the add and
reciprocal on VectorE, and sqrt(z) = exp(0.5*ln(z)).

Compute-engine APs need 32-aligned partition bases (matmul operands only
{0,32,64} and lhsT/rhs must share a base), so k is packed as 3 groups of
12/12/8 m-tiles and q is replicated into each group's partitions.
"""
import sys

import numpy as np

try:
    import concourse.bass as bass  # noqa: F401
except ImportError:
    sys.path.insert(0, "/opt/trn_rl_repo")

from contextlib import ExitStack

import concourse.bass as bass
import concourse.bacc as bacc
import concourse.mybir as mybir
from concourse.bass_utils import run_bass_kernel_spmd
from concourse.tile import TileContext

F32 = mybir.dt.float32
AF = mybir.ActivationFunctionType
OP = mybir.AluOpType

B, C, H, W = 4, 64, 64, 64
HW = H * W
QK = 8
N2 = HW // 2  # positions per core
NT = N2 // 512  # 4 n-tiles per core
MT = HW // 128  # 32 m-tiles

_BRANCH = np.full(25, 0, np.int32)
for _i in [0, 1, 2, 3, 4, 5, 8, 9, 10, 11, 12, 13, 19, 20]:
    _BRANCH[_i] = 0
for _i in [6, 7, 15, 16]:
    _BRANCH[_i] = 1
_BRANCH[17] = 2
for _i in [14, 18, 21, 22, 23, 24]:
    _BRANCH[_i] = 3

DEBUG_TAPS = False


def _build_bass():
    nc = bacc.Bacc("TRN2", target_bir_lowering=False, debug=False, num_devices=8)

    def din(name, shape):
        return nc.dram_tensor(name, list(shape), F32, kind="ExternalInput")

    x_d = din("x", [64, HW])
    xn_d = din("xn", [64, N2])
    xhalo_d = din("xhalo", [64, 128])
    wqsT_d = din("wqsT", [64, QK])
    bqs_d = din("bqs", [QK, 1])
    wkT_d = din("wkT", [64, QK])
    bk8_d = din("bk8", [QK, 1])
    wvT0b_d = din("wvT0b", [65, 65])
    c1wT_d = din("c1wT", [64, 9, 64])
    c1b_d = din("c1b", [64, 1])
    c2wT_d = din("c2wT", [64, 9, 128])
    c2b_d = din("c2b", [128, 1])
    fcTs_d = din("fcTs", [128, 25])
    fcb_d = din("fcb", [1, 25])
    BmatT_d = din("BmatT", [25, 4])
    F1cT_d = din("F1cT", [64, 64])
    F2rT_d = din("F2rT", [64, 64])
    F2iT_d = din("F2iT", [64, 64])
    nF2iT_d = din("nF2iT", [64, 64])
    id128_d = din("id128", [128, 128])
    sw_d = din("sw", [1, 1])

    y_d = nc.dram_tensor("y", [64, N2], F32, kind="ExternalOutput")
    dbg = {}
    if DEBUG_TAPS:
        dbg = {
            "d_maps": nc.dram_tensor("d_maps", [4, N2], F32, kind="ExternalOutput"),
            "d_selw": nc.dram_tensor("d_selw", [1, N2], F32, kind="ExternalOutput"),
        }

    with TileContext(nc) as tc, ExitStack() as ctx:
        sing = ctx.enter_context(tc.tile_pool(name="sing", bufs=1))
        sexp_pool = ctx.enter_context(tc.tile_pool(name="sexp", bufs=2))
        fin = ctx.enter_context(tc.tile_pool(name="fin", bufs=4))
        small = ctx.enter_context(tc.tile_pool(name="small", bufs=2))
        psA = ctx.enter_context(tc.tile_pool(name="psA", bufs=2, space="PSUM"))
        psO = ctx.enter_context(tc.tile_pool(name="psO", bufs=2, space="PSUM"))
        psB = ctx.enter_context(tc.tile_pool(name="psB", bufs=2, space="PSUM"))

        def sload(d, shape):
            t = sing.tile(list(shape), F32, tag=d.name + "_s")
            nc.sync.dma_start(out=t, in_=d.ap())
            return t

        # x augmented with a ones row (folds v-bias + denominator ones row
        # into the vT matmul)
        xau = sing.tile([65, HW], F32, tag="xau")
        nc.sync.dma_start(out=xau[0:64, :], in_=x_d.ap())
        nc.vector.memset(xau[64:65, :], 1.0)
        sxn = sload(xn_d, [64, N2])
        sxhalo = sload(xhalo_d, [64, 128])
        swqsT = sload(wqsT_d, [64, QK])
        sbqs = sload(bqs_d, [QK, 1])
        swkT = sload(wkT_d, [64, QK])
        sbk8 = sload(bk8_d, [QK, 1])
        swvT0b = sload(wvT0b_d, [65, 65])
        sc1wT = sload(c1wT_d, [64, 9, 64])
        sc1b = sload(c1b_d, [64, 1])
        sc2wT = sload(c2wT_d, [64, 9, 128])
        sc2b = sload(c2b_d, [128, 1])
        sfcTs = sload(fcTs_d, [128, 25])
        sfcb = sload(fcb_d, [1, 25])
        sBmatT = sload(BmatT_d, [25, 4])
        sF1cT = sload(F1cT_d, [64, 64])
        sF2rT = sload(F2rT_d, [64, 64])
        sF2iT = sload(F2iT_d, [64, 64])
        snF2iT = sload(nF2iT_d, [64, 64])
        sid = sload(id128_d, [128, 128])
        ssw = sload(sw_d, [1, 1])

        sx = xau[0:64, :]

        ones64 = sing.tile([64, 1], F32, tag="ones64")
        nc.vector.memset(ones64, 1.0)
        ones11 = sing.tile([1, 1], F32, tag="ones11")
        nc.vector.memset(ones11, 1.0)
        onesr = sing.tile([1, 128], F32, tag="onesr")
        nc.vector.memset(onesr, 1.0)
        eps64 = sing.tile([64, 1], F32, tag="eps64")
        nc.vector.memset(eps64, 1e-30)

        # ---------------- projections ----------------
        def kgrp(m):
            g = m // 12
            return g, m - g * 12

        kpack = sing.tile([96, 12 * 128], F32, tag="kpack")
        for j in range(HW // 512):
            pk = psB.tile([QK, 512], F32, tag="psb")
            nc.tensor.matmul(pk, swkT, sx[:, j * 512:(j + 1) * 512],
                             start=True, stop=True)
            g = j // 3
            col = (j - 3 * g) * 512
            nc.scalar.activation(kpack[32 * g:32 * g + QK, col:col + 512], pk,
                                 AF.Identity, bias=sbk8)
        qrep = sing.tile([96, N2], F32, tag="qrep")
        for t in range(NT):
            pq = psB.tile([QK, 512], F32, tag="psb")
            nc.tensor.matmul(pq, swqsT, sxn[:, t * 512:(t + 1) * 512],
                             start=True, stop=True)
            for g in range(3):
                nc.scalar.activation(qrep[32 * g:32 * g + QK,
                                          t * 512:(t + 1) * 512], pq,
                                     AF.Identity, bias=sbqs)
        # vT[m, c'] via augmented x; 7 m-tiles per PSUM bank per copy-out
        vT = sing.tile([128, MT, 65], F32, tag="vT")
        m0 = 0
        while m0 < MT:
            nb = min(7, MT - m0)
            pv = psB.tile([128, 455], F32, tag="psb")
            for i in range(nb):
                m = m0 + i
                nc.tensor.matmul(pv[:, i * 65:(i + 1) * 65],
                                 xau[:, m * 128:(m + 1) * 128], swvT0b,
                                 start=True, stop=True)
            nc.vector.tensor_copy(vT[:, m0:m0 + nb, :], pv[:, :nb * 65])
            m0 += nb

        # ------------- phases to weave into the attention loop -------------
        maps4 = sing.tile([4, N2], F32, tag="maps4")

        def sigmoid_row_to(dst_ap, src_ap, scale):
            srow = small.tile([1, 512], F32, tag="srow")
            nc.scalar.activation(srow, src_ap, AF.Exp, scale=-scale)
            nc.vector.tensor_scalar_add(srow, srow, 1.0)
            nc.vector.reciprocal(srow, srow)
            nc.sync.dma_start(out=dst_ap, in_=srow)

        cls_out = {}
        sob_state = {}

        def gen_classifier():
            xpad = sing.tile([64, 66, 66], F32, tag="pad")
            nc.gpsimd.memset(xpad[:, 0, :], 0.0)
            nc.gpsimd.memset(xpad[:, 65, :], 0.0)
            nc.gpsimd.memset(xpad[:, 1:65, 0:1], 0.0)
            nc.gpsimd.memset(xpad[:, 1:65, 65:66], 0.0)
            nc.sync.dma_start(out=xpad[:, 1:65, 1:65],
                              in_=sx.rearrange("c (h w) -> c h w", h=64))
            yield
            f1 = sing.tile([64, 32, 32], F32, tag="f1")
            for hhalf in range(2):
                pc1 = psB.tile([64, 512], F32, tag="psb")
                for kk in range(9):
                    dy, dx = kk // 3, kk % 3
                    rhs = bass.AP(
                        tensor=xpad.tensor,
                        offset=xpad.offset + (2 * (hhalf * 16) + dy) * 66 + dx,
                        ap=[list(xpad.ap[0]), [2 * 66, 16], [2, 32]],
                    )
                    nc.tensor.matmul(pc1, sc1wT[:, kk, :], rhs,
                                     start=(kk == 0), stop=(kk == 8))
                    yield
                nc.scalar.activation(
                    f1[:, hhalf * 16:(hhalf + 1) * 16, :],
                    pc1.rearrange("c (h w) -> c h w", h=16),
                    AF.Relu, bias=sc1b)
                yield
            f1pad = sing.tile([64, 34, 34], F32, tag="pad")
            nc.gpsimd.memset(f1pad[:, 0, :], 0.0)
            nc.gpsimd.memset(f1pad[:, 33, :], 0.0)
            nc.gpsimd.memset(f1pad[:, 1:33, 0:1], 0.0)
            nc.gpsimd.memset(f1pad[:, 1:33, 33:34], 0.0)
            nc.vector.tensor_copy(f1pad[:, 1:33, 1:33], f1)
            yield
            f2 = sing.tile([128, 256], F32, tag="f2")
            pc2 = psB.tile([128, 256], F32, tag="psb")
            for kk in range(9):
                dy, dx = kk // 3, kk % 3
                rhs = bass.AP(
                    tensor=f1pad.tensor,
                    offset=f1pad.offset + dy * 34 + dx,
                    ap=[list(f1pad.ap[0]), [2 * 34, 16], [2, 16]],
                )
                nc.tensor.matmul(pc2, sc2wT[:, kk, :], rhs,
                                 start=(kk == 0), stop=(kk == 8))
                yield
            nc.scalar.activation(f2, pc2, AF.Relu, bias=sc2b)
            feat = small.tile([128, 1], F32, tag="feat")
            nc.vector.reduce_sum(feat, f2, axis=mybir.AxisListType.X)
            plog = psB.tile([1, 25], F32, tag="psb")
            nc.tensor.matmul(plog, feat, sfcTs, start=True, stop=True)
            yield
            lg = small.tile([1, 25], F32, tag="lg")
            nc.vector.tensor_add(lg, plog, sfcb)
            mx1 = small.tile([1, 1], F32, tag="mx1")
            nc.vector.reduce_max(mx1, lg, axis=mybir.AxisListType.X)
            eq = small.tile([1, 25], F32, tag="eq")
            nc.vector.tensor_scalar(eq, lg, mx1, None, op0=OP.is_ge)
            eqs = small.tile([1, 1], F32, tag="eqs")
            nc.vector.reduce_sum(eqs, eq, axis=mybir.AxisListType.X)
            eqr = small.tile([1, 1], F32, tag="eqr")
            nc.vector.reciprocal(eqr, eqs)
            nc.vector.tensor_scalar_mul(eq, eq, eqr)
            peqT = psB.tile([25, 1], F32, tag="psb")
            nc.tensor.matmul(peqT, eq, ones11, start=True, stop=True)
            yield
            eqT = small.tile([25, 1], F32, tag="eqT")
            nc.vector.tensor_copy(eqT, peqT)
            poh = psB.tile([4, 1], F32, tag="psb")
            nc.tensor.matmul(poh, sBmatT, eqT, start=True, stop=True)
            oh = small.tile([4, 1], F32, tag="oh")
            nc.vector.tensor_copy(oh, poh)
            cls_out["oh"] = oh

        def gen_hist():
            for t in range(NT):
                ph = psB.tile([1, 512], F32, tag="psb")
                nc.tensor.matmul(ph, ones64, sxn[:, t * 512:(t + 1) * 512],
                                 start=True, stop=True)
                yield
                sigmoid_row_to(maps4[2:3, t * 512:(t + 1) * 512], ph,
                               1.0 / 64.0)
                yield

        def gen_sobel():
            xsob = sing.tile([64, 34, 66], F32, tag="pad")
            nc.gpsimd.memset(xsob[:, :, 0:1], 0.0)
            nc.gpsimd.memset(xsob[:, :, 65:66], 0.0)
            nc.sync.dma_start(out=xsob[:, 1:33, 1:65],
                              in_=sxn.rearrange("c (h w) -> c h w", h=32))
            nc.sync.dma_start(
                out=xsob[:, 0:1, 1:65],
                in_=sxhalo[:, 0:64].rearrange("c (h w) -> c h w", h=1))
            nc.sync.dma_start(
                out=xsob[:, 33:34, 1:65],
                in_=sxhalo[:, 64:128].rearrange("c (h w) -> c h w", h=1))
            yield
            st1 = sing.tile([64, 32, 66], F32, tag="sob66", bufs=2)
            nc.gpsimd.tensor_add(st1, xsob[:, 0:32, :], xsob[:, 2:34, :])
            yield
            sv = sing.tile([64, 32, 66], F32, tag="sob66", bufs=2)
            nc.vector.scalar_tensor_tensor(sv, xsob[:, 1:33, :], 2.0, st1,
                                           op0=OP.mult, op1=OP.add)
            yield
            gx = sing.tile([64, 32, 64], F32, tag="sob64", bufs=2)
            nc.vector.tensor_sub(gx, sv[:, :, 2:66], sv[:, :, 0:64])
            yield
            m2 = sing.tile([64, N2], F32, tag="m2")
            gxf = gx.rearrange("c a b -> c (a b)")
            nc.vector.tensor_mul(m2, gxf, gxf)
            yield
            sd = sing.tile([64, 32, 66], F32, tag="sob66", bufs=2)
            nc.gpsimd.tensor_sub(sd, xsob[:, 2:34, :], xsob[:, 0:32, :])
            yield
            g1 = sing.tile([64, 32, 64], F32, tag="sob64", bufs=2)
            nc.gpsimd.tensor_add(g1, sd[:, :, 0:64], sd[:, :, 2:66])
            yield
            gy = sing.tile([64, 32, 64], F32, tag="sob64", bufs=2)
            nc.vector.scalar_tensor_tensor(gy, sd[:, :, 1:65], 2.0, g1,
                                           op0=OP.mult, op1=OP.add)
            yield
            gyf = gy.rearrange("c a b -> c (a b)")
            nc.vector.tensor_mul(gyf, gyf, gyf)
            yield
            nc.vector.tensor_add(m2, m2, gyf)
            yield
            sob_state["m2"] = m2

        def gen_hsv():
            mxb = small.tile([128, 16], F32, tag="mxb")
            mnb = small.tile([128, 16], F32, tag="mnb")
            for p8 in range(2):
                pt8 = psB.tile([128, 512], F32, tag="psb")
                for kk in range(8):
                    t = p8 * 8 + kk
                    nc.tensor.transpose(pt8[:, kk * 64:(kk + 1) * 64],
                                        sxn[:, t * 128:(t + 1) * 128],
                                        sid[:64, :64])
                    yield
                pt3 = pt8.rearrange("p (a b) -> p a b", a=8)
                nc.vector.tensor_reduce(mxb[:, p8 * 8:(p8 + 1) * 8], pt3,
                                        axis=mybir.AxisListType.X, op=OP.max)
                nc.vector.tensor_reduce(mnb[:, p8 * 8:(p8 + 1) * 8], pt3,
                                        axis=mybir.AxisListType.X, op=OP.min)
                yield
            hnum = small.tile([128, 16], F32, tag="hnum")
            nc.vector.scalar_tensor_tensor(hnum, mxb, 1e-6, mnb,
                                           op0=OP.add, op1=OP.subtract)
            nc.vector.tensor_scalar_add(mxb, mxb, 1e-6)
            nc.vector.reciprocal(mxb, mxb)
            nc.vector.tensor_mul(hnum, hnum, mxb)
            pht = psB.tile([16, 128], F32, tag="psb")
            nc.tensor.transpose(pht, hnum, sid)
            yield
            hrow = small.tile([16, 128], F32, tag="hrow")
            nc.vector.tensor_copy(hrow, pht)
            nc.sync.dma_start(out=maps4[1:2, :], in_=hrow)

        fft_state = {}

        def gen_fft12():
            XT2 = sing.tile([64, 64, 64], F32, tag="XT2")
            for g in range(8):
                pxt = psB.tile([64, 512], F32, tag="psb")
                for kk in range(8):
                    kidx = g * 8 + kk
                    src = bass.AP(tensor=sx.tensor, offset=sx.offset + kidx,
                                  ap=[list(sx.ap[0]), [64, 64]])
                    nc.tensor.transpose(pxt[:, kk * 64:(kk + 1) * 64], src,
                                        sid[:64, :64])
                    yield
                nc.vector.tensor_copy(XT2[:, g * 8:(g + 1) * 8, :], pxt)
                yield
            ATr = sing.tile([64, 32, 64], F32, tag="ATr")
            ATi = sing.tile([64, 32, 64], F32, tag="ATi")
            for g in range(8):
                pa = psB.tile([64, 512], F32, tag="psb")
                for cc in range(8):
                    cidx = g * 8 + cc
                    lhsT = bass.AP(tensor=XT2.tensor,
                                   offset=XT2.offset + cidx,
                                   ap=[list(XT2.ap[0]), [64, 64]])
                    nc.tensor.matmul(pa[:, cc * 64:(cc + 1) * 64], lhsT,
                                     sF1cT, start=True, stop=True)
                    yield
                src_r = bass.AP(tensor=pa.tensor, offset=pa.offset,
                                ap=[list(pa.ap[0]), [64, 8], [1, 32]])
                dst_r = bass.AP(tensor=ATr.tensor, offset=ATr.offset + g * 8,
                                ap=[list(ATr.ap[0]), [1, 8], [64, 32]])
                nc.vector.tensor_copy(dst_r, src_r)
                src_i = bass.AP(tensor=pa.tensor, offset=pa.offset + 32,
                                ap=[list(pa.ap[0]), [64, 8], [1, 32]])
                dst_i = bass.AP(tensor=ATi.tensor, offset=ATi.offset + g * 8,
                                ap=[list(ATi.ap[0]), [1, 8], [64, 32]])
                nc.vector.tensor_copy(dst_i, src_i)
                yield
            fft_state["ATrf"] = ATr.rearrange("k a b -> k (a b)")
            fft_state["ATif"] = ATi.rearrange("k a b -> k (a b)")

        def gen_fft4():
            ATrf = fft_state["ATrf"]
            ATif = fft_state["ATif"]
            fmag2 = sing.tile([64, N2], F32, tag="XT2")
            fft_state["fmag2"] = fmag2
            for t in range(4):
                cs = slice(t * 512, (t + 1) * 512)
                pyr = psB.tile([64, 512], F32, tag="psb")
                nc.tensor.matmul(pyr, sF2rT, ATrf[:, cs],
                                 start=True, stop=False)
                yield
                nc.tensor.matmul(pyr, snF2iT, ATif[:, cs],
                                 start=False, stop=True)
                yield
                pyi = psB.tile([64, 512], F32, tag="psb")
                nc.tensor.matmul(pyi, sF2rT, ATif[:, cs],
                                 start=True, stop=False)
                yield
                nc.tensor.matmul(pyi, sF2iT, ATrf[:, cs],
                                 start=False, stop=True)
                yield
                fm2 = small.tile([64, 512], F32, tag="fm2")
                nc.vector.tensor_copy(fm2, pyr)
                nc.vector.tensor_mul(fm2, fm2, fm2)
                yield
                sq2 = small.tile([64, 512], F32, tag="sq2", bufs=1)
                nc.vector.tensor_copy(sq2, pyi)
                nc.vector.tensor_mul(sq2, sq2, sq2)
                yield
                nc.vector.tensor_add(fmag2[:, cs], fm2, sq2)
                yield

        def gen_phases():
            yield from gen_classifier()
            yield from gen_fft12()
            yield from gen_sobel()
            yield from gen_hsv()
            yield from gen_fft4()

        fill = gen_phases()

        def fill_step(k=1):
            for _ in range(k):
                next(fill, None)

        # ------- attention: double-wide S/exp, fillers woven between -------
        fin_rden = []
        fin_ot = []
        ND = MT // 2
        for t in range(NT):
            cs = slice(t * 512, (t + 1) * 512)
            pO = psO.tile([65, 512], F32, tag="pso")
            se_l = {}
            for dd in range(ND + 1):
                if dd < ND:
                    pS2 = psA.tile([128, 1024], F32, tag="psa")
                    for h in range(2):
                        m = 2 * dd + h
                        g, idx = kgrp(m)
                        nc.tensor.matmul(pS2[:, h * 512:(h + 1) * 512],
                                         kpack[32 * g:32 * g + QK,
                                               idx * 128:(idx + 1) * 128],
                                         qrep[32 * g:32 * g + QK, cs],
                                         start=True, stop=True)
                    se2 = sexp_pool.tile([128, 1024], F32, tag="se")
                    nc.scalar.activation(se2, pS2, AF.Exp)
                    se_l[dd] = se2
                    fill_step(2)
                if dd >= 1:
                    d = dd - 1
                    se2 = se_l.pop(d)
                    for h in range(2):
                        m = 2 * d + h
                        nc.tensor.matmul(pO, vT[:, m, :],
                                         se2[:, h * 512:(h + 1) * 512],
                                         start=(m == 0), stop=(m == MT - 1))
                    fill_step(1)
            rden = fin.tile([1, 512], F32, tag="rden")
            nc.vector.reciprocal(rden, pO[64:65, :])
            ot = fin.tile([64, 512], F32, tag="ot")
            nc.vector.tensor_copy(ot, pO[0:64, :])
            fin_rden.append(rden)
            fin_ot.append(ot)
        for _ in fill:
            pass

        # ---- post-attention tail: transcendentals, means, sigmoids ----
        for _ in gen_hist():
            pass
        m2 = sob_state["m2"]
        nc.scalar.activation(m2, m2, AF.Ln, bias=eps64)
        nc.scalar.activation(m2, m2, AF.Exp, scale=0.5)
        for t in range(NT):
            ps = psB.tile([1, 512], F32, tag="psb")
            nc.tensor.matmul(ps, ones64, m2[:, t * 512:(t + 1) * 512],
                             start=True, stop=True)
            sigmoid_row_to(maps4[0:1, t * 512:(t + 1) * 512], ps, 1.0 / 64.0)
        fmag2 = fft_state["fmag2"]
        nc.scalar.activation(fmag2, fmag2, AF.Ln, bias=eps64)
        nc.scalar.activation(fmag2, fmag2, AF.Exp, scale=0.5)
        mapji = small.tile([64, 32], F32, tag="mapji")
        nc.vector.tensor_reduce(
            mapji, fmag2.rearrange("j (i c) -> j i c", c=64),
            axis=mybir.AxisListType.X, op=OP.add)
        nc.scalar.activation(mapji, mapji, AF.Exp, scale=-1.0 / 64.0)
        nc.vector.tensor_scalar_add(mapji, mapji, 1.0)
        nc.vector.reciprocal(mapji, mapji)
        pmt = psB.tile([32, 64], F32, tag="psb")
        nc.tensor.transpose(pmt, mapji, sid[:64, :64])
        mapij = small.tile([32, 64], F32, tag="mapij")
        nc.vector.tensor_copy(mapij, pmt)
        nc.sync.dma_start(out=maps4[3:4, :], in_=mapij)

        # blend maps by one-hot, fold in spatial_weight
        oh = cls_out["oh"]
        selw = sing.tile([1, N2], F32, tag="selw")
        for t in range(NT):
            psel = psB.tile([1, 512], F32, tag="psb")
            nc.tensor.matmul(psel, oh, maps4[:, t * 512:(t + 1) * 512],
                             start=True, stop=True)
            nc.vector.tensor_scalar_mul(selw[:, t * 512:(t + 1) * 512],
                                        psel, ssw)

        # ---------------- final combine ----------------
        for t in range(NT):
            cs = slice(t * 512, (t + 1) * 512)
            rden = fin_rden[t]
            ot = fin_ot[t]
            nc.vector.tensor_mul(rden, selw[:, cs], rden)
            pscb = psB.tile([64, 512], F32, tag="psb")
            nc.tensor.matmul(pscb, onesr[:, 0:64], rden, start=True, stop=True)
            f1t = fin.tile([64, 512], F32, tag="f1t", bufs=2)
            nc.vector.tensor_mul(f1t, ot, pscb)
            nc.vector.tensor_add(f1t, f1t, sxn[:, cs])
            nc.sync.dma_start(out=y_d[:, cs], in_=f1t)

        if DEBUG_TAPS:
            nc.sync.dma_start(out=dbg["d_maps"].ap(), in_=maps4)
            nc.sync.dma_start(out=dbg["d_selw"].ap(), in_=selw)

    nc.compile()
    return nc


_NC_CACHE = {}


def _get_nc():
    if "nc" not in _NC_CACHE:
        _NC_CACHE["nc"] = _build_bass()
    return _NC_CACHE["nc"]


def _host_in_maps(inputs):
    x = np.ascontiguousarray(np.asarray(inputs["x"], np.float32)).reshape(B, C, HW)
    wq = np.asarray(inputs["wq"], np.float32)
    bq = np.asarray(inputs["bq"], np.float32)
    wk = np.asarray(inputs["wk"], np.float32)
    bk = np.asarray(inputs["bk"], np.float32)
    wv = np.asarray(inputs["wv"], np.float32)
    bv = np.asarray(inputs["bv"], np.float32)
    c1_w = np.asarray(inputs["c1_w"], np.float32)
    c1_b = np.asarray(inputs["c1_b"], np.float32)
    c2_w = np.asarray(inputs["c2_w"], np.float32)
    c2_b = np.asarray(inputs["c2_b"], np.float32)
    fc_w = np.asarray(inputs["fc_w"], np.float32)
    fc_b = np.asarray(inputs["fc_b"], np.float32)
    sw = np.float32(np.asarray(inputs["spatial_weight"]))

    scale = np.float32(QK ** -0.5)
    wqsT = np.ascontiguousarray(wq.T * scale)
    bqs = np.ascontiguousarray((bq * scale).reshape(QK, 1))
    wkT = np.ascontiguousarray(wk.T)
    bk8 = np.ascontiguousarray(bk.reshape(QK, 1))
    wvT0b = np.zeros((65, 65), np.float32)
    wvT0b[:64, :64] = wv.T
    wvT0b[64, :64] = bv
    wvT0b[64, 64] = 1.0
    c1wT = np.ascontiguousarray(c1_w.transpose(1, 2, 3, 0).reshape(64, 9, 64))
    c1b = np.ascontiguousarray(c1_b.reshape(64, 1))
    c2wT = np.ascontiguousarray(c2_w.transpose(1, 2, 3, 0).reshape(64, 9, 128))
    c2b = np.ascontiguousarray(c2_b.reshape(128, 1))
    fcTs = np.ascontiguousarray(fc_w.T / 256.0)
    fcb = np.ascontiguousarray(fc_b.reshape(1, 25))
    BmatT = np.zeros((25, 4), np.float32)
    for l in range(25):
        BmatT[l, _BRANCH[l]] = 1.0
    Wdft = np.exp(-2j * np.pi * np.outer(np.arange(64), np.arange(64)) / 64.0)
    scols = (np.arange(64) + 32) % 64
    F2s = Wdft[scols, :]
    F2rT = np.ascontiguousarray(F2s.real.T.astype(np.float32))
    F2iT = np.ascontiguousarray(F2s.imag.T.astype(np.float32))
    nF2iT = np.ascontiguousarray(-F2iT)
    id128 = np.eye(128, dtype=np.float32)
    sw11 = np.full((1, 1), sw, np.float32)

    common = dict(wqsT=wqsT, bqs=bqs, wkT=wkT, bk8=bk8, wvT0b=wvT0b,
                  c1wT=c1wT, c1b=c1b, c2wT=c2wT, c2b=c2b, fcTs=fcTs, fcb=fcb,
                  BmatT=BmatT, F2rT=F2rT, F2iT=F2iT, nF2iT=nF2iT,
                  id128=id128, sw=sw11)

    in_maps = []
    for core in range(8):
        b, half = core // 2, core % 2
        i0, n_off = half * 32, half * N2
        xs = np.ascontiguousarray(x[b])
        xnp = np.ascontiguousarray(xs[:, n_off:n_off + N2])
        xim = xs.reshape(64, 64, 64)
        xhalo = np.zeros((64, 128), np.float32)
        if i0 > 0:
            xhalo[:, 0:64] = xim[:, i0 - 1, :]
        if i0 + 32 < 64:
            xhalo[:, 64:128] = xim[:, i0 + 32, :]
        rows = (i0 + np.arange(32) + 32) % 64
        F1s = Wdft[rows, :]
        F1cT = np.ascontiguousarray(np.concatenate(
            [F1s.real.T, F1s.imag.T], axis=1).astype(np.float32))
        im = dict(common)
        im.update(x=xs, xn=xnp, xhalo=xhalo, F1cT=F1cT)
        in_maps.append(im)
    return in_maps


def kernel(**inputs):
    nc = _get_nc()
    in_maps = _host_in_maps(inputs)
    res = run_bass_kernel_spmd(nc, in_maps, core_ids=list(range(8)))
    out = np.zeros((B, C, HW), np.float32)
    for core in range(8):
        b, half = core // 2, core % 2
        out[b, :, half * N2:(half + 1) * N2] = res.results[core]["y"]
    return out.reshape(B, C, H, W)


if __name__ == "__main__":
    d = dict(np.load("inputs.npz"))
    got = kernel(**d)
    exp = np.load("expected.npy")
    err = np.abs(got - exp)
    print("max abs err:", err.max(),
          "rel err:", err.max() / np.abs(exp).max())

